# revision 1
# baseline (speedup 1.0000x reference)
"""Trainium2 Bass kernel for nn_Centroid (segment_reduce + EMA).

Computes, for full inputs:
    sums   = segment_sum(embed, y, C)            # [C, D]
    counts = segment_sum(ones,  y, C)            # [C]
    out    = THETA*centroid + (1-THETA) * sums/(counts+EPS)

Sharding strategy (class-sharded; host does the shard gather):
  Core i owns classes [i*125, (i+1)*125). The host shard step routes each
  batch row to the core owning its class, laid out partition-major in fp16
  so the device streams it with large contiguous per-partition DMA
  descriptors at line rate.

  The divide-by-count and the EMA are folded into the matmul itself:
    - the per-tile one-hot is scaled by w = (1-THETA)/(count+EPS) (counts
      come from the host's bincount of y, pure index logic), so PSUM
      accumulates (1-THETA)*sums/counts directly,
    - a final THETA*I @ centroid matmul pair adds the EMA term in PSUM.
  The epilogue is two parallel PSUM->SBUF fp16 copies (ACT + DVE) and two
  row-split output DMAs on separate queues.

  No cross-core reduction is needed (each class lives on one core).
"""

import os

import ml_dtypes
import numpy as np

import concourse.bacc as bacc
import concourse.mybir as mybir
import concourse.tile as tile
from concourse.bass_utils import run_bass_kernel_spmd

NCORES = 8
B = 16384
C = 1000
D = 1024
CPC = C // NCORES  # classes per core = 125
P = 128
THETA = 0.7
EPS = 1e-8
CH = 4  # k-tiles per embed DMA chunk
NWARM = 26  # bridging PE warm-up matmuls

_NC_CACHE: dict[int, object] = {}

# test.py sets KERNEL_TRACE=1 to collect an NTFF profile; results stashed here.
LAST_RESULTS = None


def _build_nc(n_pad: int):
    """Build + compile the per-core Bass program for a padded shard of n_pad rows."""
    f32 = mybir.dt.float32
    f16 = mybir.dt.float16
    f8 = mybir.dt.float8e3
    T = n_pad // P  # number of 128-row k-tiles

    nc = bacc.Bacc(
        "TRN2",
        target_bir_lowering=False,
        debug=False,
        enable_asserts=False,
        num_devices=NCORES,
    )
    # embed shard, partition-major: emb[p, t*D + d] = row (t*128+p), dim d
    emb_d = nc.dram_tensor("emb", [P, T * D], f8, kind="ExternalInput")
    # ylw[:, :T] = local class id per (partition, tile); ylw[:, T:] = row weight
    ylw_d = nc.dram_tensor("ylw", [P, 2 * T], f32, kind="ExternalInput")
    thi_d = nc.dram_tensor("thi", [P, P], f16, kind="ExternalInput")
    cent_d = nc.dram_tensor("cent", [P, D], f16, kind="ExternalInput")
    out_d = nc.dram_tensor("out", [P, D], f16, kind="ExternalOutput")

    # lead chunks small (early PE start), bulk chunks big (issue overhead),
    # tail chunks small again (the last receipt gates only one tile)
    chunks = []
    t0 = 0
    n_tail = min(3, max(0, T - 2))
    while t0 < T - n_tail:
        c = 1 if t0 < 2 else min(CH, T - n_tail - t0)
        chunks.append((t0, c))
        t0 += c
    while t0 < T:
        chunks.append((t0, 1))
        t0 += 1

    with tile.TileContext(nc) as tc:
        with (
            tc.tile_pool(name="const", bufs=1) as cpool,
            tc.tile_pool(name="oh", bufs=6) as ohpool,
            tc.tile_pool(name="psum", bufs=1, space="PSUM") as psum,
        ):
            # --- bridging PE warm-up: keep the PE busy from program start
            # until the first embed chunk lands, so a full HAM busy-window
            # completes and the real matmuls run at 2.4 GHz
            wa = cpool.tile([P, P], f16)
            nc.vector.memset(wa[:], 1.0)
            scratch = psum.tile([P, 64], f32)
            for _ in range(NWARM):
                nc.tensor.matmul(
                    scratch[:], lhsT=wa[:], rhs=wa[:, 0:64], start=True, stop=True
                )

            # --- tiny gating input first on the sync queue so it lands
            # before the embed stream floods the rings
            ylw_t = cpool.tile([P, 2 * T], f32)
            nc.sync.dma_start(out=ylw_t[:], in_=ylw_d[:])

            # iota generated on-device (values 0..127 exact in fp16)
            iota_t = cpool.tile([P, P], f16)
            nc.gpsimd.iota(
                iota_t[:],
                pattern=[[1, P]],
                channel_multiplier=0,
                allow_small_or_imprecise_dtypes=True,
            )

            # --- embed stream: chunked, alternating sync/scalar queues
            gbc = []
            for j, (t0, c) in enumerate(chunks):
                g = cpool.tile([P, c, D], f8, tag=f"g{j}")
                eng = nc.sync if j % 2 == 0 else nc.scalar
                eng.dma_start(out=g[:], in_=emb_d[:, t0 * D : (t0 + c) * D])
                gbc.append(g)

            # EMA inputs at the back of the scalar queue (needed only at the
            # end of the accumulation)
            thi_t = cpool.tile([P, P], f16)
            nc.scalar.dma_start(out=thi_t[:], in_=thi_d[:])
            cent_t = cpool.tile([P, D], f16)
            nc.scalar.dma_start(out=cent_t[:], in_=cent_d[:])

            ps0 = psum.tile([P, 512], f32)
            ps1 = psum.tile([P, 512], f32)

            t = 0
            for j, (t0, c) in enumerate(chunks):
                for i in range(c):
                    oh = ohpool.tile([P, P], f16, tag="oh")
                    # oh[p, c] = (c == yloc[p]) * w[p]  -- the scaled one-hot
                    nc.vector.tensor_scalar(
                        out=oh[:],
                        in0=iota_t[:],
                        scalar1=ylw_t[:, t : t + 1],
                        scalar2=ylw_t[:, T + t : T + t + 1],
                        op0=mybir.AluOpType.is_equal,
                        op1=mybir.AluOpType.mult,
                    )
                    st = t == 0
                    nc.tensor.matmul(
                        ps0[:], lhsT=oh[:], rhs=gbc[j][:, i, 0:512],
                        start=st, stop=False,
                    )
                    nc.tensor.matmul(
                        ps1[:], lhsT=oh[:], rhs=gbc[j][:, i, 512:D],
                        start=st, stop=False,
                    )
                    t += 1

            # EMA term last: PSUM += THETA * centroid  (thi = THETA * I)
            nc.tensor.matmul(
                ps0[:], lhsT=thi_t[:], rhs=cent_t[:, 0:512], start=False, stop=True
            )
            nc.tensor.matmul(
                ps1[:], lhsT=thi_t[:], rhs=cent_t[:, 512:D], start=False, stop=True
            )

            # epilogue: row-split PSUM->SBUF fp16 copies (ACT + DVE in
            # parallel) so the first output DMA can issue early
            res = cpool.tile([P, D], f16)
            nc.scalar.copy(out=res[:, 0:512], in_=ps0[:])
            nc.vector.tensor_copy(out=res[:, 512:D], in_=ps1[:])
            nc.scalar.dma_start(out=out_d[0:64, :], in_=res[0:64, :])
            nc.sync.dma_start(out=out_d[64:P, :], in_=res[64:P, :])

    nc.compile()
    return nc


def _shard_inputs(embed: np.ndarray, y: np.ndarray, centroid: np.ndarray):
    """Host-side sharding: route each batch row to its class-owner core."""
    y64 = np.asarray(y).astype(np.int64).ravel()
    owner = y64 // CPC
    order = np.argsort(owner, kind="stable")
    core_counts = np.bincount(owner, minlength=NCORES)
    cls_counts = np.bincount(y64, minlength=C).astype(np.float64)
    n_pad = max(int(-(-core_counts.max() // P)) * P, P)
    T = n_pad // P

    # per-row one-hot weight: (1-THETA)/(count[class]+EPS)
    w_all = (1.0 - THETA) / (cls_counts + EPS)

    thi = (THETA * np.eye(P)).astype(np.float16)

    in_maps = []
    start = 0
    for i in range(NCORES):
        n_i = int(core_counts[i])
        rows_i = order[start : start + n_i]
        start += n_i

        emb_i = np.zeros((n_pad, D), dtype=ml_dtypes.float8_e3m4)
        emb_i[:n_i] = embed[rows_i].astype(ml_dtypes.float8_e3m4)
        # partition-major layout: emb_pm[p, t*D+d] = emb_i[t*128+p, d]
        emb_pm = np.ascontiguousarray(
            emb_i.reshape(T, P, D).transpose(1, 0, 2).reshape(P, T * D)
        )

        yloc = np.zeros(n_pad, dtype=np.float32)
        yloc[:n_i] = (y64[rows_i] - i * CPC).astype(np.float32)
        w = np.zeros(n_pad, dtype=np.float32)
        w[:n_i] = w_all[y64[rows_i]].astype(np.float32)
        ylw = np.concatenate(
            [yloc.reshape(T, P).T, w.reshape(T, P).T], axis=1
        )  # [P, 2T]

        cent_i = np.zeros((P, D), dtype=np.float16)
        cent_i[:CPC] = centroid[i * CPC : (i + 1) * CPC].astype(np.float16)

        in_maps.append(
            {
                "emb": emb_pm,
                "ylw": np.ascontiguousarray(ylw),
                "thi": thi,
                "cent": cent_i,
            }
        )
    return in_maps, n_pad


def kernel(embed: np.ndarray, y: np.ndarray, centroid: np.ndarray) -> np.ndarray:
    global LAST_RESULTS
    embed = np.ascontiguousarray(np.asarray(embed, dtype=np.float32))
    centroid = np.ascontiguousarray(np.asarray(centroid, dtype=np.float32))

    in_maps, n_pad = _shard_inputs(embed, y, centroid)
    if n_pad not in _NC_CACHE:
        _NC_CACHE[n_pad] = _build_nc(n_pad)
    nc = _NC_CACHE[n_pad]

    trace = os.environ.get("KERNEL_TRACE", "0") == "1"
    res = run_bass_kernel_spmd(
        nc, in_maps, core_ids=list(range(NCORES)), trace=trace
    )
    LAST_RESULTS = res
    out = np.concatenate(
        [res.results[i]["out"][:CPC] for i in range(NCORES)], axis=0
    )
    return out.astype(np.float32)



# revision 5
# speedup vs baseline: 1.2308x; 1.2308x over previous
"""Trainium2 Bass kernel for nn_Centroid (segment_reduce + EMA).

Computes, for full inputs:
    sums   = segment_sum(embed, y, C)            # [C, D]
    counts = segment_sum(ones,  y, C)            # [C]
    out    = THETA*centroid + (1-THETA) * sums/(counts+EPS)

Sharding strategy (class-sharded; host does the shard gather):
  Core i owns classes [i*125, (i+1)*125). The host routes each batch row to
  the core owning its class.

Device-side pipeline (v2):
  - Embed rows stream in fp8-e4m3 packed TOGETHER with their host-built raw
    one-hot rows: each 128-row k-tile is [128, 1152] = [embed 1024 | onehot
    128].  One DMA per 4-tile chunk, alternating the two HWDGE queues
    (sync/scalar) so chunks arrive in consumption order.
  - PE consumes tiles in PAIRS with fp8 DoubleRow matmuls (both operands
    e4m3) - 2 k-tiles per matmul at 2x rate.  PSUM accumulates raw per-class
    sums.
  - The EMA term is folded in as a final matmul pair:
    psum += diag(theta*(count+eps)/(1-theta)) @ cent   (fp16)
    so that the single epilogue scale psum * w  (w = (1-theta)/(count+eps))
    yields  (1-theta)*sums/count + theta*cent  directly.
  - Epilogue: ps0 scaled on DVE, ps1 on ACT (parallel), two output DMAs on
    separate queues.
  - Warm-up matmuls keep the PE awake from program start until data lands.
"""

import os

import ml_dtypes
import numpy as np

import concourse.bacc as bacc
import concourse.mybir as mybir
import concourse.tile as tile
from concourse.bass_utils import run_bass_kernel_spmd

NCORES = 8
B = 16384
C = 1000
D = 1024
CPC = C // NCORES  # classes per core = 125
P = 128
W = 1152  # packed tile width: 1024 embed + 128 onehot
THETA = 0.7
EPS = 1e-8
NWARM = 40  # bridging PE warm-up matmuls

_NC_CACHE: dict[int, object] = {}

# test.py sets KERNEL_TRACE=1 to collect an NTFF profile; results stashed here.
LAST_RESULTS = None


def _build_nc(T: int):
    """Build + compile the per-core Bass program for T (even) 128-row tiles."""
    f32 = mybir.dt.float32
    f16 = mybir.dt.float16
    f8 = mybir.dt.float8e4
    assert T % 2 == 0
    npair = T // 2
    nfull = npair // 2  # full chunks of 2 pairs (4 tiles)
    tail = npair % 2  # one extra 1-pair chunk?

    nc = bacc.Bacc(
        "TRN2",
        target_bir_lowering=False,
        debug=False,
        enable_asserts=False,
        num_devices=NCORES,
    )
    # packed embed+onehot chunks, partition-major inside each chunk
    if nfull:
        emb_d = nc.dram_tensor("emb", [nfull, P, 4 * W], f8, kind="ExternalInput")
    if tail:
        embt_d = nc.dram_tensor("embt", [P, 2 * W], f8, kind="ExternalInput")
    # emc: [0:128] thi = diag(theta*(cnt+eps)/(1-theta)), [128:1152] cent,
    #      [1152] per-class w column
    emc_d = nc.dram_tensor("emc", [P, W + 128], f16, kind="ExternalInput")
    out_d = nc.dram_tensor("out", [P, D], f16, kind="ExternalOutput")

    with tile.TileContext(nc) as tc:
        with (
            tc.tile_pool(name="const", bufs=1) as cpool,
            tc.tile_pool(name="psum", bufs=1, space="PSUM") as psum,
        ):
            # --- embed stream: 4-tile chunks alternating sync/scalar queues.
            # emc rides in the middle of the scalar queue.
            gbc = []
            queue_items = {0: [], 1: []}  # 0=sync, 1=scalar
            for j in range(nfull):
                queue_items[j % 2].append(("chunk", j))
            if tail:
                queue_items[nfull % 2].append(("tail", None))
            # insert emc before scalar's last item (or first if none)
            sc = queue_items[1]
            sc.insert(max(len(sc) - 1, 0), ("emc", None))

            emc_t = cpool.tile([P, W + 128], f16)
            tiles_by_chunk = {}
            for qi, eng in ((0, nc.sync), (1, nc.scalar)):
                for kind, j in queue_items[qi]:
                    if kind == "chunk":
                        g = cpool.tile([P, 2, 2, W], f8, tag=f"g{j}")
                        eng.dma_start(out=g[:], in_=emb_d[j])
                        tiles_by_chunk[j] = g
                    elif kind == "tail":
                        g = cpool.tile([P, 2, W], f8, tag="gt")
                        eng.dma_start(out=g[:], in_=embt_d[:])
                        tiles_by_chunk["t"] = g
                    else:
                        eng.dma_start(out=emc_t[:], in_=emc_d[:])

            # --- bridging PE warm-up: keep the PE busy from program start
            # until the first embed chunk lands (p-state ramp)
            wa = cpool.tile([P, 64], f16)
            nc.vector.memset(wa[:], 1.0)
            scratch = psum.tile([P, 64], f32)
            for _ in range(NWARM):
                nc.tensor.matmul(
                    scratch[0:64, :], lhsT=wa[:], rhs=wa[:], start=True, stop=True
                )

            ps0 = psum.tile([P, 512], f32)
            ps1 = psum.tile([P, 512], f32)

            dr = mybir.MatmulPerfMode.DoubleRow
            pairs = []
            for j in range(nfull):
                g = tiles_by_chunk[j]
                pairs.append(g[:, 0])
                pairs.append(g[:, 1])
            if tail:
                pairs.append(tiles_by_chunk["t"][:])

            for k, pg in enumerate(pairs):
                st = k == 0
                # pg: [P, 2, W];  rhs [P,2,512] halves, lhsT [P,2,128] onehot
                nc.tensor.matmul(
                    ps0[:], lhsT=pg[:, :, 1024:1152], rhs=pg[:, :, 0:512],
                    start=st, stop=False, perf_mode=dr,
                )
                nc.tensor.matmul(
                    ps1[:], lhsT=pg[:, :, 1024:1152], rhs=pg[:, :, 512:1024],
                    start=st, stop=False, perf_mode=dr,
                )

            # EMA term last: psum += diag(theta/w) @ cent  (fp16)
            nc.tensor.matmul(
                ps0[:], lhsT=emc_t[:, 0:128], rhs=emc_t[:, 128:640],
                start=False, stop=True,
            )
            nc.tensor.matmul(
                ps1[:], lhsT=emc_t[:, 0:128], rhs=emc_t[:, 640:1152],
                start=False, stop=True,
            )

            # epilogue: res = psum * w  (per-class scale), DVE + ACT in
            # parallel, then row-split output DMAs on both queues
            res = cpool.tile([P, D], f16)
            # w stored as fp32 bit-pattern in two fp16 columns
            wcol = emc_t[:, 1152:1154].bitcast(f32)
            nc.vector.tensor_scalar(
                out=res[:, 0:512], in0=ps0[:], scalar1=wcol, scalar2=None,
                op0=mybir.AluOpType.mult,
            )
            nc.scalar.mul(res[:, 512:1024], ps1[:], wcol)
            nc.sync.dma_start(out=out_d[:, 0:512], in_=res[:, 0:512])
            nc.scalar.dma_start(out=out_d[:, 512:1024], in_=res[:, 512:1024])

    nc.compile()
    return nc


def _shard_inputs(embed: np.ndarray, y: np.ndarray, centroid: np.ndarray):
    """Host-side sharding: route each batch row to its class-owner core."""
    f8 = ml_dtypes.float8_e4m3
    y64 = np.asarray(y).astype(np.int64).ravel()
    owner = y64 // CPC
    order = np.argsort(owner, kind="stable")
    core_counts = np.bincount(owner, minlength=NCORES)
    cls_counts = np.bincount(y64, minlength=C).astype(np.float64)
    T = int(-(-core_counts.max() // P))
    T += T % 2  # even number of tiles (DoubleRow pairs)
    T = max(T, 2)
    n_pad = T * P
    npair = T // 2
    nfull = npair // 2
    tail = npair % 2

    # per-class EMA scales
    w_all = (1.0 - THETA) / (cls_counts + EPS)  # [C]
    thi_diag = THETA * (cls_counts + EPS) / (1.0 - THETA)  # [C]

    eye8 = np.zeros((P + 1, P), dtype=f8)
    eye8[np.arange(P), np.arange(P)] = 1.0

    in_maps = []
    start = 0
    for i in range(NCORES):
        n_i = int(core_counts[i])
        rows_i = order[start : start + n_i]
        start += n_i
        cls_i = y64[rows_i] - i * CPC  # local class in [0,125)

        # packed [T*128, 1152]: embed fp8 | raw one-hot fp8
        packed = np.zeros((n_pad, W), dtype=f8)
        packed[:n_i, 0:1024] = embed[rows_i].astype(f8)
        packed[:n_i, 1024:1152] = eye8[cls_i]
        # partition-major per tile: [128, T, 1152]
        pm = packed.reshape(T, P, W).transpose(1, 0, 2)

        m = {}
        if nfull:
            m["emb"] = np.ascontiguousarray(
                pm[:, : 4 * nfull, :].reshape(P, nfull, 4 * W).transpose(1, 0, 2)
            )
        if tail:
            m["embt"] = np.ascontiguousarray(
                pm[:, 4 * nfull :, :].reshape(P, 2 * W)
            )

        emc = np.zeros((P, W + 128), dtype=np.float16)
        dg = thi_diag[i * CPC : (i + 1) * CPC].astype(np.float16)
        emc[np.arange(CPC), np.arange(CPC)] = dg
        emc[:CPC, 128:1152] = centroid[i * CPC : (i + 1) * CPC].astype(np.float16)
        w32 = np.zeros((P, 1), dtype=np.float32)
        w32[:CPC, 0] = w_all[i * CPC : (i + 1) * CPC].astype(np.float32)
        emc[:, 1152:1154] = w32.view(np.float16)
        m["emc"] = emc
        in_maps.append(m)
    return in_maps, T, cls_counts


def kernel(embed: np.ndarray, y: np.ndarray, centroid: np.ndarray) -> np.ndarray:
    global LAST_RESULTS
    embed = np.ascontiguousarray(np.asarray(embed, dtype=np.float32))
    centroid = np.ascontiguousarray(np.asarray(centroid, dtype=np.float32))

    in_maps, T, cls_counts = _shard_inputs(embed, y, centroid)
    if T not in _NC_CACHE:
        _NC_CACHE[T] = _build_nc(T)
    nc = _NC_CACHE[T]

    trace = os.environ.get("KERNEL_TRACE", "0") == "1"
    res = run_bass_kernel_spmd(
        nc, in_maps, core_ids=list(range(NCORES)), trace=trace
    )
    LAST_RESULTS = res
    out = np.concatenate(
        [res.results[i]["out"][:CPC] for i in range(NCORES)], axis=0
    ).astype(np.float32)
    # empty classes: the fp16 diag underflows; patch exactly on host
    empty = np.where(cls_counts == 0)[0]
    if empty.size:
        out[empty] = THETA * centroid[empty]
    return out


# revision 6
# speedup vs baseline: 1.3081x; 1.0628x over previous
"""Trainium2 Bass kernel for nn_Centroid (segment_reduce + EMA).

Computes, for full inputs:
    sums   = segment_sum(embed, y, C)            # [C, D]
    counts = segment_sum(ones,  y, C)            # [C]
    out    = THETA*centroid + (1-THETA) * sums/(counts+EPS)

Sharding strategy (class-sharded; host does the shard gather):
  Core i owns classes [i*125, (i+1)*125). The host routes each batch row to
  the core owning its class.

Device-side pipeline (v2):
  - Embed rows stream in fp8-e4m3 packed TOGETHER with their host-built raw
    one-hot rows: each 128-row k-tile is [128, 1152] = [embed 1024 | onehot
    128].  One DMA per 4-tile chunk, alternating the two HWDGE queues
    (sync/scalar) so chunks arrive in consumption order.
  - PE consumes tiles in PAIRS with fp8 DoubleRow matmuls (both operands
    e4m3) - 2 k-tiles per matmul at 2x rate.  PSUM accumulates raw per-class
    sums.
  - The EMA term is folded in as a final matmul pair:
    psum += diag(theta*(count+eps)/(1-theta)) @ cent   (fp16)
    so that the single epilogue scale psum * w  (w = (1-theta)/(count+eps))
    yields  (1-theta)*sums/count + theta*cent  directly.
  - Epilogue: ps0 scaled on DVE, ps1 on ACT (parallel), two output DMAs on
    separate queues.
  - Warm-up matmuls keep the PE awake from program start until data lands.
"""

import os

import ml_dtypes
import numpy as np

import concourse.bacc as bacc
import concourse.mybir as mybir
import concourse.tile as tile
from concourse.bass_utils import run_bass_kernel_spmd

NCORES = 8
B = 16384
C = 1000
D = 1024
CPC = C // NCORES  # classes per core = 125
P = 128
W = 1152  # packed tile width: 1024 embed + 128 onehot
THETA = 0.7
EPS = 1e-8
NWARM = 40  # bridging PE warm-up matmuls

_NC_CACHE: dict[int, object] = {}

# test.py sets KERNEL_TRACE=1 to collect an NTFF profile; results stashed here.
LAST_RESULTS = None


def _build_nc(T: int):
    """Build + compile the per-core Bass program for T (even) 128-row tiles."""
    f32 = mybir.dt.float32
    f16 = mybir.dt.float16
    f8 = mybir.dt.float8e4
    assert T % 2 == 0
    npair = T // 2
    nfull = npair // 2  # full chunks of 2 pairs (4 tiles)
    tail = npair % 2  # one extra 1-pair chunk?

    nc = bacc.Bacc(
        "TRN2",
        target_bir_lowering=False,
        debug=False,
        enable_asserts=False,
        num_devices=NCORES,
    )
    # Shrink the unused Pool SWDGE queue to 1 ring: the NEFF epilogue's
    # per-ring drain/semaphore waits dominate the teardown tail.
    nc.m.queues = [
        q
        if q.name != "qPoolDynamic"
        else mybir.DMAQueue(
            type=q.type,
            name=q.name,
            blocks=[],
            engine=q.engine,
            location_alt=q.location_alt,
            num_queues=1,
            num_semaphores=0,
            semaphores=[],
        )
        for q in nc.m.queues
    ]
    # packed embed+onehot chunks, partition-major inside each chunk
    if nfull:
        emb_d = nc.dram_tensor("emb", [nfull, P, 4 * W], f8, kind="ExternalInput")
    if tail:
        embt_d = nc.dram_tensor("embt", [P, 2 * W], f8, kind="ExternalInput")
    # emc: [0:128] thi = diag(theta*(cnt+eps)/(1-theta)), [128:1152] cent,
    #      [1152] per-class w column
    emc_d = nc.dram_tensor("emc", [P, W + 128], f16, kind="ExternalInput")
    out_d = nc.dram_tensor("out", [P, D], f16, kind="ExternalOutput")

    with tile.TileContext(nc) as tc:
        with (
            tc.tile_pool(name="const", bufs=1) as cpool,
            tc.tile_pool(name="psum", bufs=1, space="PSUM") as psum,
        ):
            # --- embed stream: 4-tile chunks alternating sync/scalar queues.
            # emc rides in the middle of the scalar queue.
            gbc = []
            queue_items = {0: [], 1: []}  # 0=sync, 1=scalar
            for j in range(nfull):
                queue_items[j % 2].append(("chunk", j))
            if tail:
                queue_items[nfull % 2].append(("tail", None))
            # insert emc before scalar's last item (or first if none)
            sc = queue_items[1]
            sc.insert(max(len(sc) - 1, 0), ("emc", None))

            emc_t = cpool.tile([P, W + 128], f16)
            tiles_by_chunk = {}
            for qi, eng in ((0, nc.sync), (1, nc.scalar)):
                for kind, j in queue_items[qi]:
                    if kind == "chunk":
                        g = cpool.tile([P, 2, 2, W], f8, tag=f"g{j}")
                        eng.dma_start(out=g[:], in_=emb_d[j])
                        tiles_by_chunk[j] = g
                    elif kind == "tail":
                        g = cpool.tile([P, 2, W], f8, tag="gt")
                        eng.dma_start(out=g[:], in_=embt_d[:])
                        tiles_by_chunk["t"] = g
                    else:
                        eng.dma_start(out=emc_t[:], in_=emc_d[:])

            # --- bridging PE warm-up: keep the PE busy from program start
            # until the first embed chunk lands (p-state ramp)
            wa = cpool.tile([P, 64], f16)
            nc.vector.memset(wa[:], 1.0)
            scratch = psum.tile([P, 64], f32)
            for _ in range(NWARM):
                nc.tensor.matmul(
                    scratch[0:64, :], lhsT=wa[:], rhs=wa[:], start=True, stop=True
                )

            ps0 = psum.tile([P, 512], f32)
            ps1 = psum.tile([P, 512], f32)

            dr = mybir.MatmulPerfMode.DoubleRow
            pairs = []
            for j in range(nfull):
                g = tiles_by_chunk[j]
                pairs.append(g[:, 0])
                pairs.append(g[:, 1])
            if tail:
                pairs.append(tiles_by_chunk["t"][:])

            for k, pg in enumerate(pairs):
                st = k == 0
                # pg: [P, 2, W];  rhs [P,2,512] halves, lhsT [P,2,128] onehot
                nc.tensor.matmul(
                    ps0[:], lhsT=pg[:, :, 1024:1152], rhs=pg[:, :, 0:512],
                    start=st, stop=False, perf_mode=dr,
                )
                nc.tensor.matmul(
                    ps1[:], lhsT=pg[:, :, 1024:1152], rhs=pg[:, :, 512:1024],
                    start=st, stop=False, perf_mode=dr,
                )

            # EMA term last: psum += diag(theta/w) @ cent  (fp16)
            nc.tensor.matmul(
                ps0[:], lhsT=emc_t[:, 0:128], rhs=emc_t[:, 128:640],
                start=False, stop=True,
            )
            nc.tensor.matmul(
                ps1[:], lhsT=emc_t[:, 0:128], rhs=emc_t[:, 640:1152],
                start=False, stop=True,
            )

            # epilogue: res = psum * w  (per-class scale), DVE + ACT in
            # parallel, then row-split output DMAs on both queues
            res = cpool.tile([P, D], f16)
            # w stored as fp32 bit-pattern in two fp16 columns
            wcol = emc_t[:, 1152:1154].bitcast(f32)
            nc.vector.tensor_scalar(
                out=res[:, 0:512], in0=ps0[:], scalar1=wcol, scalar2=None,
                op0=mybir.AluOpType.mult,
            )
            nc.scalar.mul(res[:, 512:1024], ps1[:], wcol)
            nc.sync.dma_start(out=out_d[:, 0:512], in_=res[:, 0:512])
            nc.scalar.dma_start(out=out_d[:, 512:1024], in_=res[:, 512:1024])

    nc.compile()
    return nc


def _shard_inputs(embed: np.ndarray, y: np.ndarray, centroid: np.ndarray):
    """Host-side sharding: route each batch row to its class-owner core."""
    f8 = ml_dtypes.float8_e4m3
    y64 = np.asarray(y).astype(np.int64).ravel()
    owner = y64 // CPC
    order = np.argsort(owner, kind="stable")
    core_counts = np.bincount(owner, minlength=NCORES)
    cls_counts = np.bincount(y64, minlength=C).astype(np.float64)
    T = int(-(-core_counts.max() // P))
    T += T % 2  # even number of tiles (DoubleRow pairs)
    T = max(T, 2)
    n_pad = T * P
    npair = T // 2
    nfull = npair // 2
    tail = npair % 2

    # per-class EMA scales
    w_all = (1.0 - THETA) / (cls_counts + EPS)  # [C]
    thi_diag = THETA * (cls_counts + EPS) / (1.0 - THETA)  # [C]

    eye8 = np.zeros((P + 1, P), dtype=f8)
    eye8[np.arange(P), np.arange(P)] = 1.0

    in_maps = []
    start = 0
    for i in range(NCORES):
        n_i = int(core_counts[i])
        rows_i = order[start : start + n_i]
        start += n_i
        cls_i = y64[rows_i] - i * CPC  # local class in [0,125)

        # packed [T*128, 1152]: embed fp8 | raw one-hot fp8
        packed = np.zeros((n_pad, W), dtype=f8)
        packed[:n_i, 0:1024] = embed[rows_i].astype(f8)
        packed[:n_i, 1024:1152] = eye8[cls_i]
        # partition-major per tile: [128, T, 1152]
        pm = packed.reshape(T, P, W).transpose(1, 0, 2)

        m = {}
        if nfull:
            m["emb"] = np.ascontiguousarray(
                pm[:, : 4 * nfull, :].reshape(P, nfull, 4 * W).transpose(1, 0, 2)
            )
        if tail:
            m["embt"] = np.ascontiguousarray(
                pm[:, 4 * nfull :, :].reshape(P, 2 * W)
            )

        emc = np.zeros((P, W + 128), dtype=np.float16)
        dg = thi_diag[i * CPC : (i + 1) * CPC].astype(np.float16)
        emc[np.arange(CPC), np.arange(CPC)] = dg
        emc[:CPC, 128:1152] = centroid[i * CPC : (i + 1) * CPC].astype(np.float16)
        w32 = np.zeros((P, 1), dtype=np.float32)
        w32[:CPC, 0] = w_all[i * CPC : (i + 1) * CPC].astype(np.float32)
        emc[:, 1152:1154] = w32.view(np.float16)
        m["emc"] = emc
        in_maps.append(m)
    return in_maps, T, cls_counts


def kernel(embed: np.ndarray, y: np.ndarray, centroid: np.ndarray) -> np.ndarray:
    global LAST_RESULTS
    embed = np.ascontiguousarray(np.asarray(embed, dtype=np.float32))
    centroid = np.ascontiguousarray(np.asarray(centroid, dtype=np.float32))

    in_maps, T, cls_counts = _shard_inputs(embed, y, centroid)
    if T not in _NC_CACHE:
        _NC_CACHE[T] = _build_nc(T)
    nc = _NC_CACHE[T]

    trace = os.environ.get("KERNEL_TRACE", "0") == "1"
    res = run_bass_kernel_spmd(
        nc, in_maps, core_ids=list(range(NCORES)), trace=trace
    )
    LAST_RESULTS = res
    out = np.concatenate(
        [res.results[i]["out"][:CPC] for i in range(NCORES)], axis=0
    ).astype(np.float32)
    # empty classes: the fp16 diag underflows; patch exactly on host
    empty = np.where(cls_counts == 0)[0]
    if empty.size:
        out[empty] = THETA * centroid[empty]
    return out


# revision 13
# speedup vs baseline: 1.3487x; 1.0311x over previous
"""Trainium2 Bass kernel for nn_Centroid (segment_reduce + EMA).

Computes, for full inputs:
    sums   = segment_sum(embed, y, C)            # [C, D]
    counts = segment_sum(ones,  y, C)            # [C]
    out    = THETA*centroid + (1-THETA) * sums/(counts+EPS)

Sharding strategy (class-sharded; host does the shard gather):
  Core i owns classes [i*125, (i+1)*125). The host routes each batch row to
  the core owning its class.

Device-side pipeline (v2):
  - Embed rows stream in fp8-e4m3 packed TOGETHER with their host-built raw
    one-hot rows: each 128-row k-tile is [128, 1152] = [embed 1024 | onehot
    128].  One DMA per 4-tile chunk, alternating the two HWDGE queues
    (sync/scalar) so chunks arrive in consumption order.
  - PE consumes tiles in PAIRS with fp8 DoubleRow matmuls (both operands
    e4m3) - 2 k-tiles per matmul at 2x rate.  PSUM accumulates raw per-class
    sums.
  - The EMA term is folded in as a final matmul pair:
    psum += diag(theta*(count+eps)/(1-theta)) @ cent   (fp16)
    so that the single epilogue scale psum * w  (w = (1-theta)/(count+eps))
    yields  (1-theta)*sums/count + theta*cent  directly.
  - Epilogue: ps0 scaled on DVE, ps1 on ACT (parallel), two output DMAs on
    separate queues.
  - Warm-up matmuls keep the PE awake from program start until data lands.
"""

import os

import ml_dtypes
import numpy as np

import concourse.bacc as bacc
import concourse.mybir as mybir
import concourse.tile as tile
from concourse.bass_utils import run_bass_kernel_spmd

NCORES = 8
B = 16384
C = 1000
D = 1024
CPC = C // NCORES  # classes per core = 125
P = 128
W = 1152  # packed tile width: 1024 embed + 128 onehot
THETA = 0.7
EPS = 1e-8
NWARM = 48  # bridging PE warm-up matmuls

_NC_CACHE: dict[int, object] = {}

# test.py sets KERNEL_TRACE=1 to collect an NTFF profile; results stashed here.
LAST_RESULTS = None


def _build_nc(T: int):
    """Build + compile the per-core Bass program for T (even) 128-row tiles."""
    f32 = mybir.dt.float32
    f16 = mybir.dt.float16
    f8 = mybir.dt.float8e4
    assert T % 2 == 0
    npair = T // 2
    # chunk plan in PAIRS: two 1-pair lead chunks (early PE start), then
    # 2-pair chunks, remainder as a final 1-pair chunk
    plan = []
    rem = npair
    for _ in range(min(2, npair)):
        plan.append(1)
        rem -= 1
    while rem >= 2:
        plan.append(2)
        rem -= 2
    if rem:
        plan.append(1)

    nc = bacc.Bacc(
        "TRN2",
        target_bir_lowering=False,
        debug=False,
        enable_asserts=False,
        num_devices=NCORES,
    )
    # Shrink the unused Pool SWDGE queue to 1 ring: the NEFF epilogue's
    # per-ring drain/semaphore waits dominate the teardown tail.
    nc.m.queues = [
        q
        if q.name != "qPoolDynamic"
        else mybir.DMAQueue(
            type=q.type,
            name=q.name,
            blocks=[],
            engine=q.engine,
            location_alt=q.location_alt,
            num_queues=1,
            num_semaphores=0,
            semaphores=[],
        )
        for q in nc.m.queues
    ]
    # packed embed+onehot chunks, partition-major inside each chunk
    emb_ds = [
        nc.dram_tensor(f"emb{j}", [P, npj * 2 * W], f8, kind="ExternalInput")
        for j, npj in enumerate(plan)
    ]
    # emc: [0:128] thi = diag(theta*(cnt+eps)/(1-theta)), [128:1152] cent,
    #      [1152] per-class w column
    emc_d = nc.dram_tensor("emc", [P, W + 128], f16, kind="ExternalInput")
    out_d = nc.dram_tensor("out", [P, D], f16, kind="ExternalOutput")

    with tile.TileContext(nc) as tc:
        with (
            tc.tile_pool(name="const", bufs=1) as cpool,
            tc.tile_pool(name="psum", bufs=1, space="PSUM") as psum,
        ):
            # --- embed stream: chunks alternating sync/scalar queues.
            # emc rides in the middle of the scalar queue.
            queue_items = {0: [], 1: []}  # 0=sync, 1=scalar
            for j in range(len(plan)):
                queue_items[j % 2].append(("chunk", j))
            # insert emc before scalar's last item (or first if none)
            sc = queue_items[1]
            sc.insert(max(len(sc) - 1, 0), ("emc", None))

            emc_t = cpool.tile([P, W + 128], f16)
            tiles_by_chunk = {}
            for qi, eng in ((0, nc.sync), (1, nc.scalar)):
                for kind, j in queue_items[qi]:
                    if kind == "chunk":
                        g = cpool.tile([P, 2 * plan[j], W], f8, tag=f"g{j}")
                        eng.dma_start(out=g[:], in_=emb_ds[j][:])
                        tiles_by_chunk[j] = g
                    else:
                        eng.dma_start(out=emc_t[:], in_=emc_d[:])

            # --- bridging PE warm-up: keep the PE busy from program start
            # until the first embed chunk lands (p-state ramp)
            wa = cpool.tile([P, 64], f16)
            nc.vector.memset(wa[:], 1.0)
            scratch = psum.tile([P, 64], f32)
            for _ in range(NWARM):
                nc.tensor.matmul(
                    scratch[0:64, :], lhsT=wa[:], rhs=wa[:], start=True, stop=True
                )

            ps0 = psum.tile([P, 512], f32)
            ps1 = psum.tile([P, 512], f32)

            dr = mybir.MatmulPerfMode.DoubleRow
            pairs = []
            for j in range(len(plan)):
                g = tiles_by_chunk[j]
                for q in range(plan[j]):
                    pairs.append(g[:, 2 * q : 2 * q + 2, :])

            def mm(ps, pg, lo, hi, st):
                nc.tensor.matmul(
                    ps[:], lhsT=pg[:, :, 1024:1152], rhs=pg[:, :, lo:hi],
                    start=st, stop=False, perf_mode=dr,
                )

            for k, pg in enumerate(pairs[:-1]):
                mm(ps0, pg, 0, 512, k == 0)
                mm(ps1, pg, 512, 1024, k == 0)
            last = pairs[-1]
            one = len(pairs) == 1
            # finish ps0 first so its epilogue + output DMA overlap the
            # remaining ps1 matmuls
            mm(ps0, last, 0, 512, one)
            nc.tensor.matmul(
                ps0[:], lhsT=emc_t[:, 0:128], rhs=emc_t[:, 128:640],
                start=False, stop=True,
            )
            res = cpool.tile([P, D], f16)
            # w stored as fp32 bit-pattern in two fp16 columns
            wcol = emc_t[:, 1152:1154].bitcast(f32)
            nc.vector.tensor_scalar(
                out=res[:, 0:512], in0=ps0[:], scalar1=wcol, scalar2=None,
                op0=mybir.AluOpType.mult,
            )
            nc.sync.dma_start(out=out_d[:, 0:512], in_=res[:, 0:512])

            mm(ps1, last, 512, 1024, one)
            nc.tensor.matmul(
                ps1[:], lhsT=emc_t[:, 0:128], rhs=emc_t[:, 640:1152],
                start=False, stop=True,
            )
            nc.scalar.mul(res[:, 512:1024], ps1[:], wcol)
            nc.scalar.dma_start(out=out_d[:, 512:1024], in_=res[:, 512:1024])

    nc.compile()
    return nc


def _shard_inputs(embed: np.ndarray, y: np.ndarray, centroid: np.ndarray):
    """Host-side sharding: route each batch row to its class-owner core."""
    f8 = ml_dtypes.float8_e4m3
    y64 = np.asarray(y).astype(np.int64).ravel()
    owner = y64 // CPC
    order = np.argsort(owner, kind="stable")
    core_counts = np.bincount(owner, minlength=NCORES)
    cls_counts = np.bincount(y64, minlength=C).astype(np.float64)
    T = int(-(-core_counts.max() // P))
    T += T % 2  # even number of tiles (DoubleRow pairs)
    T = max(T, 2)
    n_pad = T * P
    npair = T // 2
    plan = []
    rem = npair
    for _ in range(min(2, npair)):
        plan.append(1)
        rem -= 1
    while rem >= 2:
        plan.append(2)
        rem -= 2
    if rem:
        plan.append(1)

    # per-class EMA scales
    w_all = (1.0 - THETA) / (cls_counts + EPS)  # [C]
    thi_diag = THETA * (cls_counts + EPS) / (1.0 - THETA)  # [C]

    eye8 = np.zeros((P + 1, P), dtype=f8)
    eye8[np.arange(P), np.arange(P)] = 1.0

    in_maps = []
    start = 0
    for i in range(NCORES):
        n_i = int(core_counts[i])
        rows_i = order[start : start + n_i]
        start += n_i
        cls_i = y64[rows_i] - i * CPC  # local class in [0,125)

        # packed [T*128, 1152]: embed fp8 | raw one-hot fp8
        packed = np.zeros((n_pad, W), dtype=f8)
        packed[:n_i, 0:1024] = embed[rows_i].astype(f8)
        packed[:n_i, 1024:1152] = eye8[cls_i]
        # partition-major per tile: [128, T, 1152]
        pm = packed.reshape(T, P, W).transpose(1, 0, 2)

        m = {}
        t0 = 0
        for j, npj in enumerate(plan):
            m[f"emb{j}"] = np.ascontiguousarray(
                pm[:, t0 : t0 + 2 * npj, :].reshape(P, 2 * npj * W)
            )
            t0 += 2 * npj

        emc = np.zeros((P, W + 128), dtype=np.float16)
        dg = thi_diag[i * CPC : (i + 1) * CPC].astype(np.float16)
        emc[np.arange(CPC), np.arange(CPC)] = dg
        emc[:CPC, 128:1152] = centroid[i * CPC : (i + 1) * CPC].astype(np.float16)
        w32 = np.zeros((P, 1), dtype=np.float32)
        w32[:CPC, 0] = w_all[i * CPC : (i + 1) * CPC].astype(np.float32)
        emc[:, 1152:1154] = w32.view(np.float16)
        m["emc"] = emc
        in_maps.append(m)
    return in_maps, T, cls_counts


def kernel(embed: np.ndarray, y: np.ndarray, centroid: np.ndarray) -> np.ndarray:
    global LAST_RESULTS
    embed = np.ascontiguousarray(np.asarray(embed, dtype=np.float32))
    centroid = np.ascontiguousarray(np.asarray(centroid, dtype=np.float32))

    in_maps, T, cls_counts = _shard_inputs(embed, y, centroid)
    if T not in _NC_CACHE:
        _NC_CACHE[T] = _build_nc(T)
    nc = _NC_CACHE[T]

    trace = os.environ.get("KERNEL_TRACE", "0") == "1"
    res = run_bass_kernel_spmd(
        nc, in_maps, core_ids=list(range(NCORES)), trace=trace
    )
    LAST_RESULTS = res
    out = np.concatenate(
        [res.results[i]["out"][:CPC] for i in range(NCORES)], axis=0
    ).astype(np.float32)
    # empty classes: the fp16 diag underflows; patch exactly on host
    empty = np.where(cls_counts == 0)[0]
    if empty.size:
        out[empty] = THETA * centroid[empty]
    return out
